# revision 1
# baseline (speedup 1.0000x reference)
"""Trainium2 Bass kernel for BertLinearSelfAttention (linear attention).

Reference computation (per batch b, head h):
    q,k,v = X @ W{q,k,v} + b{q,k,v}            # [S, D] -> heads of 64
    qf, kf = elu(q)+1, elu(k)+1                # = min(exp(x),1) + max(x,0)
    kv[d,e]  = sum_s kf[s,d] v[s,e]            # [64, 64]
    ksum[d]  = sum_s kf[s,d]
    out[s,e] = (sum_d qf[s,d] kv[d,e]) / (sum_d qf[s,d] ksum[d])

Sharding: 8 cores = (4 batches) x (2 head-groups of 8 heads / 512 proj cols).
X is fed pre-transposed ([D, S], contraction dim on partitions) and weights in
their natural [D, CG] layout, both declared fp32r so they stream straight from
HBM into the PE with no on-device transpose or rounding pass.

All matmuls run in fp32r (single "HIGH" pass, full PE rate, ~2^-13 rounding).
Pass A: k/v projections + feature maps + kv/ksum accumulation per 512-token
chunk. Pass B: q^T projection + block-diagonal numerator/denominator matmuls
+ divide. The PE stream is software-pipelined: consumers of DVE/ACT results
(kv of chunk i, num of chunk j) are emitted one chunk late so the PE never
stalls on the elementwise chains (keeps the HAM clock at 2.4 GHz).
"""

import os
import sys

import numpy as np

_REPO = "/opt/trn_rl_repo"
if os.path.isdir(_REPO) and _REPO not in sys.path:
    sys.path.insert(0, _REPO)

B, S, D, H, HD = 4, 4096, 1024, 16, 64
NCORES = 8
CG = 512            # projection columns per core (8 heads)
NH = CG // HD       # 8 heads per core
HE = HD + 2         # head cols incl ksum column + even-pad (fp32r needs even N)
CHUNK = 512         # tokens per chunk
NSUB = CHUNK // 128     # 4 token sub-tiles per chunk
NCHUNK = S // CHUNK     # 8 chunks
NKT = D // 128          # 8 contraction tiles
P = 128

_CACHED_NC = None


def _build():
    import concourse.tile as tile
    from concourse import bacc, mybir
    from contextlib import ExitStack

    F32 = mybir.dt.float32
    F32R = mybir.dt.float32r
    Alu = mybir.AluOpType
    Act = mybir.ActivationFunctionType

    nc = bacc.Bacc("TRN2", target_bir_lowering=False, debug=False,
                   num_devices=NCORES)

    xt_d = nc.dram_tensor("xt", [D, S], F32R, kind="ExternalInput").ap()
    w_d = {
        "q": nc.dram_tensor("wq", [D, CG], F32R, kind="ExternalInput").ap(),
        "k": nc.dram_tensor("wk", [D, CG], F32R, kind="ExternalInput").ap(),
        "v": nc.dram_tensor("wv", [D, CG], F32R, kind="ExternalInput").ap(),
    }
    bq_d = nc.dram_tensor("bq", [CG], F32, kind="ExternalInput").ap()
    bk_d = nc.dram_tensor("bk", [1, CG], F32R, kind="ExternalInput").ap()
    bv_d = nc.dram_tensor("bv", [1, CG], F32, kind="ExternalInput").ap()
    ones_d = nc.dram_tensor("onesr", [1, P], F32R, kind="ExternalInput").ap()
    out_d = nc.dram_tensor("out", [S, CG], F32, kind="ExternalOutput").ap()

    with tile.TileContext(nc) as tc:
        with ExitStack() as ctx:
            const = ctx.enter_context(tc.tile_pool(name="const", bufs=1))
            wpool = ctx.enter_context(tc.tile_pool(name="wpool", bufs=1))
            xtpool = ctx.enter_context(tc.tile_pool(name="xtpool", bufs=14))

            def load_xt(ci):
                tok0 = ci * CHUNK
                xt = []
                for kt in range(NKT):
                    t = xtpool.tile([P, CHUNK], F32R, tag="xt", name="xt")
                    nc.sync.dma_start(
                        t[:], xt_d[kt * P:(kt + 1) * P, tok0:tok0 + CHUNK])
                    xt.append(t)
                return xt

            # queue the first chunk's X^T ahead of all setup DMAs
            xt0 = load_xt(0)

            # ---- constants / weights (one-time) ----
            ones_r = const.tile([1, P], F32R, tag="onesr")
            nc.sync.dma_start(ones_r[:], ones_d[:])
            bk_r = const.tile([1, CG], F32R, tag="bkr")
            nc.sync.dma_start(bk_r[:], bk_d[:])

            # q bias per-partition: bq_sb[:, ct] = bq[ct*128:(ct+1)*128]
            bq_sb = const.tile([P, CG // P], F32, tag="bqsb")
            nc.sync.dma_start(bq_sb[:], bq_d.rearrange("(c p) -> p c", p=P))

            # tail columns for V': [1.0 (ksum), 0.0 (pad)] per head
            ones_col = const.tile([P, NH * 2], F32, tag="onescol")
            nc.vector.memset(ones_col[:], 0.0)
            nc.vector.memset(
                ones_col[:].rearrange("p (h e) -> p h e", e=2)[:, :, 0:1], 1.0)

            # v bias replicated to all partitions (added during V' evict)
            bv32 = const.tile([1, CG], F32, tag="bv32")
            nc.sync.dma_start(bv32[:], bv_d[:])
            bv_rep = const.tile([P, CG], F32, tag="bvrep")
            nc.gpsimd.partition_broadcast(bv_rep[:], bv32[:])

            # weights, fp32r straight from DRAM (gpsimd queue; keeps the sync
            # queue free for the first X^T tiles)
            w_r = {}
            for nm in ("k", "v", "q"):
                w_r[nm] = wpool.tile([P, NKT * CG], F32R, tag=f"w{nm}r",
                                     name=f"w{nm}r")
                for kt in range(NKT):
                    nc.gpsimd.dma_start(w_r[nm][:, kt * CG:(kt + 1) * CG],
                                        w_d[nm][kt * P:(kt + 1) * P, :])

            # kv + ksum accumulator (SBUF side, f32; feeds the kvblocks)
            kv_sb = wpool.tile([HD, NH * HE], F32, tag="kvsb")
            nc.vector.memset(kv_sb[:], 0.0)
            # block-diagonal kv per c-tile: rows 0:64 = head 2ct (cols 0:HE),
            # rows 64:128 = head 2ct+1 (cols HE:2HE); zeros elsewhere.
            # Lets the num matmul use the full K=128 array per c-tile.
            kvblocks = [wpool.tile([P, 2 * HE], F32R, tag=f"kvb{i}",
                                   name=f"kvb{i}") for i in range(CG // P)]

            kfpool = ctx.enter_context(tc.tile_pool(name="kfpool", bufs=9))
            vppool = ctx.enter_context(tc.tile_pool(name="vppool", bufs=9))
            qftpool = ctx.enter_context(tc.tile_pool(name="qftpool", bufs=9))
            tmp = ctx.enter_context(tc.tile_pool(name="tmp", bufs=8))
            outpool = ctx.enter_context(tc.tile_pool(name="outp", bufs=6))
            rcpool = ctx.enter_context(tc.tile_pool(name="rcp", bufs=16))
            pps = ctx.enter_context(
                tc.tile_pool(name="pps", bufs=4, space="PSUM"))
            sps = ctx.enter_context(
                tc.tile_pool(name="sps", bufs=4, space="PSUM"))

            kf_c = {}   # chunk -> list of kf tiles (per sub)
            vp_c = {}
            qft_c = {}  # chunk -> list of q_feat^T tiles (per ctile)

            def a_chunk(ci, xt=None):
                """Pass A for chunk ci: k/v projections + feature maps."""
                if xt is None:
                    xt = load_xt(ci)
                kfs, vps = [], []
                for nm in ("k", "v"):
                    for sub in range(NSUB):
                        ps = pps.tile([P, CG], F32, tag="pps", name="pps")
                        for kt in range(NKT):
                            nc.tensor.matmul(
                                ps[:],
                                xt[kt][:, sub * P:(sub + 1) * P],
                                w_r[nm][:, kt * CG:(kt + 1) * CG],
                                start=(kt == 0),
                                stop=(nm == "v" and kt == NKT - 1))
                        if nm == "k":
                            # + bias via K=1 matmul
                            nc.tensor.matmul(ps[:], ones_r[:], bk_r[:],
                                             start=False, stop=True)
                            # kf = min(exp(k),1) + max(k,0)   (fp32r out)
                            e = tmp.tile([P, CG], F32, tag="t", name="t_e")
                            nc.scalar.activation(e[:], ps[:], Act.Exp)
                            m = tmp.tile([P, CG], F32, tag="t", name="t_m")
                            nc.vector.tensor_scalar(
                                m[:], e[:], 1.0, None, Alu.min)
                            r = tmp.tile([P, CG], F32, tag="t", name="t_r")
                            nc.vector.tensor_scalar(
                                r[:], ps[:], 0.0, None, Alu.max)
                            kf = kfpool.tile([P, CG], F32R, tag="kf",
                                             name="kf")
                            nc.vector.tensor_tensor(kf[:], m[:], r[:], Alu.add)
                            kfs.append(kf)
                        else:
                            # V' = [v + bv | 1 | 0] per head (fp32r out)
                            vp = vppool.tile([P, NH * HE], F32R, tag="vp",
                                             name="vp")
                            nc.vector.tensor_tensor(
                                vp[:].rearrange(
                                    "p (h e) -> p h e", e=HE)[:, :, :HD],
                                ps[:].rearrange("p (h e) -> p h e", e=HD),
                                bv_rep[:].rearrange(
                                    "p (h e) -> p h e", e=HD),
                                Alu.add)
                            nc.vector.tensor_copy(
                                vp[:].rearrange(
                                    "p (h e) -> p h e", e=HE)[:, :, HD:],
                                ones_col[:].rearrange(
                                    "p (h e) -> p h e", e=2))
                            vps.append(vp)
                kf_c[ci] = kfs
                vp_c[ci] = vps

            def a_kv(ci):
                """kv/ksum accumulation for chunk ci (one bank per head)."""
                kfs, vps = kf_c.pop(ci), vp_c.pop(ci)
                for h in range(NH):
                    kvt = sps.tile([HD, HE], F32, tag="sps", name="kvt")
                    for sub in range(NSUB):
                        nc.tensor.matmul(
                            kvt[:],
                            kfs[sub][:, h * HD:(h + 1) * HD],
                            vps[sub][:, h * HE:(h + 1) * HE],
                            start=(sub == 0), stop=(sub == NSUB - 1))
                    acc = kv_sb[:, h * HE:(h + 1) * HE]
                    nc.vector.tensor_tensor(acc, acc, kvt[:], Alu.add)

            def b_chunk(cj):
                """Pass B for chunk cj: q^T projection + feature map."""
                xtb = load_xt(cj)
                qft = []
                for ct in range(CG // P):
                    ps = pps.tile([P, CHUNK], F32, tag="pps", name="qps")
                    for kt in range(NKT):
                        nc.tensor.matmul(
                            ps[:],
                            w_r["q"][:, kt * CG + ct * P: kt * CG + (ct + 1) * P],
                            xtb[kt][:],
                            start=(kt == 0), stop=(kt == NKT - 1))
                    bcol = bq_sb[:, ct:ct + 1]
                    e = tmp.tile([P, CHUNK], F32, tag="t", name="t_qe")
                    nc.scalar.activation(e[:], ps[:], Act.Exp, bias=bcol)
                    m = tmp.tile([P, CHUNK], F32, tag="t", name="t_qm")
                    nc.vector.tensor_scalar(m[:], e[:], 1.0, None, Alu.min)
                    r = tmp.tile([P, CHUNK], F32, tag="t", name="t_qr")
                    nc.vector.tensor_scalar(
                        r[:], ps[:], bcol, 0.0, Alu.add, Alu.max)
                    qf = qftpool.tile([P, CHUNK], F32R, tag="qft", name="qft")
                    nc.vector.tensor_tensor(qf[:], m[:], r[:], Alu.add)
                    qft.append(qf)
                qft_c[cj] = qft

            def b_num(cj):
                """num/den matmuls + divide + store for chunk cj."""
                tok0 = cj * CHUNK
                qft = qft_c.pop(cj)
                outs = [outpool.tile([P, CG], F32, tag="out", name=f"osb{i}")
                        for i in range(NSUB)]
                for sub in range(NSUB):
                    for ct in range(CG // P):
                        # [num|den|pad] for heads (2ct, 2ct+1) in one matmul
                        pn = sps.tile([P, 2 * HE], F32, tag="sps", name="pn")
                        nc.tensor.matmul(
                            pn[:],
                            qft[ct][:, sub * P:(sub + 1) * P],
                            kvblocks[ct][:],
                            start=True, stop=True)
                        rc = rcpool.tile([P, 2], F32, tag="rc", name="rc")
                        nc.vector.reciprocal(
                            rc[:].rearrange("p (h e) -> p h e", e=1),
                            pn[:].rearrange(
                                "p (h e) -> p h e", e=HE)[:, :, HD:HD + 1])
                        # out = num * (1/den), per-partition scale on ACT
                        for half in range(2):
                            nc.scalar.mul(
                                outs[sub][:, (2 * ct + half) * HD:
                                          (2 * ct + half + 1) * HD],
                                pn[:, half * HE:half * HE + HD],
                                rc[:, half:half + 1])
                for sub in range(NSUB):
                    nc.sync.dma_start(
                        out_d[tok0 + sub * P: tok0 + (sub + 1) * P, :],
                        outs[sub][:])

            # ---- software-pipelined stream ----
            for ci in range(NCHUNK):
                a_chunk(ci, xt0 if ci == 0 else None)
                if ci >= 1:
                    a_kv(ci - 1)
            b_chunk(0)          # q^T needs no kv; bridges the A->B gap
            a_kv(NCHUNK - 1)
            # kv complete -> build block-diagonal fp32r kvblocks
            for ct in range(CG // P):
                kstg = outpool.tile([P, 2 * HE], F32, tag="out", name="kstg")
                nc.vector.memset(kstg[:], 0.0)
                nc.vector.tensor_copy(
                    kstg[0:HD, 0:HE],
                    kv_sb[:, (2 * ct) * HE:(2 * ct + 1) * HE])
                nc.vector.tensor_copy(
                    kstg[HD:P, HE:2 * HE],
                    kv_sb[:, (2 * ct + 1) * HE:(2 * ct + 2) * HE])
                nc.vector.tensor_copy(kvblocks[ct][:], kstg[:])
            for cj in range(1, NCHUNK):
                b_chunk(cj)
                b_num(cj - 1)
            b_num(NCHUNK - 1)

    nc.compile()
    return nc


def _get_nc():
    global _CACHED_NC
    if _CACHED_NC is None:
        _CACHED_NC = _build()
    return _CACHED_NC


def _make_in_maps(hidden_states, Wq, bq, Wk, bk, Wv, bv):
    hs = np.asarray(hidden_states, np.float32)
    ones = np.ones((1, P), np.float32)
    arrs = {"wq": np.asarray(Wq, np.float32), "wk": np.asarray(Wk, np.float32),
            "wv": np.asarray(Wv, np.float32), "bq": np.asarray(bq, np.float32),
            "bk": np.asarray(bk, np.float32), "bv": np.asarray(bv, np.float32)}
    xts = [np.ascontiguousarray(hs[b].T) for b in range(B)]
    in_maps = []
    for c in range(NCORES):
        b, g = divmod(c, 2)
        sl = slice(g * CG, (g + 1) * CG)
        in_maps.append({
            "xt": xts[b],
            "wq": np.ascontiguousarray(arrs["wq"][:, sl]),
            "wk": np.ascontiguousarray(arrs["wk"][:, sl]),
            "wv": np.ascontiguousarray(arrs["wv"][:, sl]),
            "bq": np.ascontiguousarray(arrs["bq"][sl]),
            "bk": np.ascontiguousarray(arrs["bk"][sl]).reshape(1, CG),
            "bv": np.ascontiguousarray(arrs["bv"][sl]).reshape(1, CG),
            "onesr": ones,
        })
    return in_maps


def _run(in_maps, **kwargs):
    from concourse.bass_utils import run_bass_kernel_spmd
    nc = _get_nc()
    return run_bass_kernel_spmd(nc, in_maps, core_ids=list(range(NCORES)),
                                **kwargs)


def _assemble(results):
    out = np.empty((B, S, D), np.float32)
    for c in range(NCORES):
        b, g = divmod(c, 2)
        out[b, :, g * CG:(g + 1) * CG] = results[c]["out"]
    return out


def kernel(hidden_states, Wq, bq, Wk, bk, Wv, bv):
    in_maps = _make_in_maps(hidden_states, Wq, bq, Wk, bk, Wv, bv)
    res = _run(in_maps)
    return _assemble(res.results)



# revision 3
# speedup vs baseline: 1.1895x; 1.1895x over previous
"""Trainium2 Bass kernel for BertLinearSelfAttention (linear attention).

Reference computation (per batch b, head h):
    q,k,v = X @ W{q,k,v} + b{q,k,v}            # [S, D] -> heads of 64
    qf, kf = elu(q)+1, elu(k)+1                # = min(exp(x),1) + max(x,0)
    kv[d,e]  = sum_s kf[s,d] v[s,e]            # [64, 64]
    ksum[d]  = sum_s kf[s,d]
    out[s,e] = (sum_d qf[s,d] kv[d,e]) / (sum_d qf[s,d] ksum[d])

Sharding: 8 cores = (4 batches) x (2 head-groups of 8 heads / 512 proj cols).

All matmuls in bf16 (1 col/cycle, same PE rate as fp32r; the 2e-2 rel-err
gate leaves ~50x headroom over bf16's ~0.5% noise). Single pass over X:
per 512-token chunk compute k/v/q projections + feature maps, accumulate
kv/ksum, and stash q-features (bf16, 4.2MB) in SBUF. The v bias is folded
into kv afterwards as the rank-1 update ksum x bv, so the v path needs only
a PSUM->SBUF copy (on ACT). Tail: per chunk, block-diagonal num/den matmuls
+ reciprocal + broadcast multiply (split DVE/ACT/GPS) + bf16 store.
"""

import os
import sys

import numpy as np
import ml_dtypes

_REPO = "/opt/trn_rl_repo"
if os.path.isdir(_REPO) and _REPO not in sys.path:
    sys.path.insert(0, _REPO)

B, S, D, H, HD = 4, 4096, 1024, 16, 64
NCORES = 8
CG = 512            # projection columns per core (8 heads)
NH = CG // HD       # 8 heads per core
HE = HD + 2         # head cols incl ksum column + pad
CHUNK = 512         # tokens per chunk
NSUB = CHUNK // 128     # 4 token sub-tiles per chunk
NCHUNK = S // CHUNK     # 8 chunks
NKT = D // 128          # 8 contraction tiles
P = 128
NCT = CG // P           # 4 column tiles (2 heads each)

BF16 = ml_dtypes.bfloat16

_CACHED_NC = None


def _build():
    import concourse.tile as tile
    from concourse import bacc, mybir
    from contextlib import ExitStack

    F32 = mybir.dt.float32
    BF = mybir.dt.bfloat16
    Alu = mybir.AluOpType
    Act = mybir.ActivationFunctionType

    nc = bacc.Bacc("TRN2", target_bir_lowering=False, debug=False,
                   num_devices=NCORES)

    xt_d = nc.dram_tensor("xt", [D, S], BF, kind="ExternalInput").ap()
    wk_d = nc.dram_tensor("wk", [D, CG], BF, kind="ExternalInput").ap()
    wv_d = nc.dram_tensor("wv", [D, CG], BF, kind="ExternalInput").ap()
    wq_d = nc.dram_tensor("wq", [D, CG], BF, kind="ExternalInput").ap()
    bq_d = nc.dram_tensor("bq", [CG], F32, kind="ExternalInput").ap()
    bk_d = nc.dram_tensor("bk", [1, CG], BF, kind="ExternalInput").ap()
    bv_d = nc.dram_tensor("bv", [1, NH * HD], F32, kind="ExternalInput").ap()
    ones_d = nc.dram_tensor("onesb", [1, P], BF, kind="ExternalInput").ap()
    out_d = nc.dram_tensor("out", [S, CG], BF, kind="ExternalOutput").ap()

    with tile.TileContext(nc) as tc:
        with ExitStack() as ctx:
            const = ctx.enter_context(tc.tile_pool(name="const", bufs=1))
            wpool = ctx.enter_context(tc.tile_pool(name="wpool", bufs=1))
            xtpool = ctx.enter_context(tc.tile_pool(name="xtpool", bufs=18))

            def load_xt(ci):
                tok0 = ci * CHUNK
                xt = []
                for kt in range(NKT):
                    t = xtpool.tile([P, CHUNK], BF, tag="xt", name="xt")
                    nc.sync.dma_start(
                        t[:], xt_d[kt * P:(kt + 1) * P, tok0:tok0 + CHUNK])
                    xt.append(t)
                return xt

            # first chunk's X^T ahead of everything on the sync queue
            xt0 = load_xt(0)

            # weights on the gpsimd queue, k first so chunk 0 starts early
            w_sb = {}
            for nm, wd in (("k", wk_d), ("v", wv_d), ("q", wq_d)):
                w_sb[nm] = wpool.tile([P, NKT * CG], BF, tag=f"w{nm}",
                                      name=f"w{nm}")
                for kt in range(NKT):
                    nc.gpsimd.dma_start(w_sb[nm][:, kt * CG:(kt + 1) * CG],
                                        wd[kt * P:(kt + 1) * P, :])

            # ---- small constants ----
            ones_r = const.tile([1, P], BF, tag="onesb")
            nc.sync.dma_start(ones_r[:], ones_d[:])
            bk_r = const.tile([1, CG], BF, tag="bkr")
            nc.sync.dma_start(bk_r[:], bk_d[:])
            bq_sb = const.tile([P, NCT], F32, tag="bqsb")
            nc.sync.dma_start(bq_sb[:], bq_d.rearrange("(c p) -> p c", p=P))
            bv_sb = const.tile([1, NH * HD], F32, tag="bv32")
            nc.sync.dma_start(bv_sb[:], bv_d[:])
            bv_rep = const.tile([P, NH * HD], F32, tag="bvrep")
            nc.gpsimd.partition_broadcast(bv_rep[:], bv_sb[:])

            # kv accumulator (SBUF f32): per head [64, HE] (ksum in col HD)
            kv_sb = wpool.tile([HD, NH * HE], F32, tag="kvsb")
            nc.vector.memset(kv_sb[:], 0.0)

            # block-diagonal num weights [128,128] per ct + den cols [128,2]
            kvbn = [wpool.tile([P, P], BF, tag=f"kvbn{i}", name=f"kvbn{i}")
                    for i in range(NCT)]
            kvbd = [wpool.tile([P, 2], BF, tag=f"kvbd{i}", name=f"kvbd{i}")
                    for i in range(NCT)]
            for t in kvbn + kvbd:
                nc.vector.memset(t[:], 0.0)

            # persistent V' tiles (2 chunks' worth): tail cols preset once
            vp_tiles = [wpool.tile([P, NH * HE], BF, tag=f"vp{i}",
                                   name=f"vp{i}") for i in range(2 * NSUB)]
            for t in vp_tiles:
                nc.vector.memset(
                    t[:].rearrange("p (h e) -> p h e", e=HE)[:, :, HD:], 0.0)
                nc.vector.memset(
                    t[:].rearrange("p (h e) -> p h e", e=HE)[:, :, HD:HD + 1],
                    1.0)

            # q-feature store for the whole sequence (bf16, 4.2MB)
            qft_all = wpool.tile([P, NCHUNK * NCT * CHUNK], BF, tag="qft")

            kfpool = ctx.enter_context(tc.tile_pool(name="kfpool", bufs=6))
            tmp = ctx.enter_context(tc.tile_pool(name="tmp", bufs=10))
            outpool = ctx.enter_context(tc.tile_pool(name="outp", bufs=6))
            rcpool = ctx.enter_context(tc.tile_pool(name="rcp", bufs=8))
            pps = ctx.enter_context(
                tc.tile_pool(name="pps", bufs=5, space="PSUM"))
            sps = ctx.enter_context(
                tc.tile_pool(name="sps", bufs=3, space="PSUM"))

            def do_chunk(ci, xt):
                kfs = []
                # ---- k projection + feature map ----
                for sub in range(NSUB):
                    ps = pps.tile([P, CG], F32, tag="pps", name="kps")
                    for kt in range(NKT):
                        nc.tensor.matmul(
                            ps[:],
                            xt[kt][:, sub * P:(sub + 1) * P],
                            w_sb["k"][:, kt * CG:(kt + 1) * CG],
                            start=(kt == 0), stop=False)
                    nc.tensor.matmul(ps[:], ones_r[:], bk_r[:],
                                     start=False, stop=True)
                    e = tmp.tile([P, CG], BF, tag="t", name="t_e")
                    nc.scalar.activation(e[:], ps[:], Act.Exp)
                    m = tmp.tile([P, CG], BF, tag="t", name="t_m")
                    nc.vector.tensor_scalar(m[:], e[:], 1.0, None, Alu.min)
                    kf = kfpool.tile([P, CG], BF, tag="kf", name="kf")
                    # kf = max(ps,0) + m
                    nc.vector.scalar_tensor_tensor(
                        kf[:], ps[:], 0.0, m[:], Alu.max, Alu.add)
                    kfs.append(kf)
                # ---- v projection (no bias; folded into kv later) ----
                vps = []
                for sub in range(NSUB):
                    ps = pps.tile([P, CG], F32, tag="pps", name="vps")
                    for kt in range(NKT):
                        nc.tensor.matmul(
                            ps[:],
                            xt[kt][:, sub * P:(sub + 1) * P],
                            w_sb["v"][:, kt * CG:(kt + 1) * CG],
                            start=(kt == 0), stop=(kt == NKT - 1))
                    vp = vp_tiles[(ci % 2) * NSUB + sub]
                    nc.scalar.copy(
                        vp[:].rearrange("p (h e) -> p h e", e=HE)[:, :, :HD],
                        ps[:].rearrange("p (h e) -> p h e", e=HD))
                    vps.append(vp)
                # ---- q^T projection + feature map (stored) ----
                for ct in range(NCT):
                    ps = pps.tile([P, CHUNK], F32, tag="pps", name="qps")
                    for kt in range(NKT):
                        nc.tensor.matmul(
                            ps[:],
                            w_sb["q"][:, kt * CG + ct * P:
                                      kt * CG + (ct + 1) * P],
                            xt[kt][:],
                            start=(kt == 0), stop=(kt == NKT - 1))
                    bcol = bq_sb[:, ct:ct + 1]
                    e = tmp.tile([P, CHUNK], BF, tag="t", name="t_qe")
                    nc.scalar.activation(e[:], ps[:], Act.Exp, bias=bcol)
                    m = tmp.tile([P, CHUNK], BF, tag="t", name="t_qm")
                    nc.vector.tensor_scalar(m[:], e[:], 1.0, None, Alu.min)
                    r = tmp.tile([P, CHUNK], BF, tag="t", name="t_qr")
                    nc.vector.tensor_scalar(
                        r[:], ps[:], bcol, 0.0, Alu.add, Alu.max)
                    q0 = (ci * NCT + ct) * CHUNK
                    nc.vector.tensor_tensor(
                        qft_all[:, q0:q0 + CHUNK], m[:], r[:], Alu.add)
                # ---- kv accumulation (head pairs: M=128, N=2*HE) ----
                for hp in range(NH // 2):
                    kvt = sps.tile([P, 2 * HE], F32, tag="sps", name="kvt")
                    for sub in range(NSUB):
                        nc.tensor.matmul(
                            kvt[:],
                            kfs[sub][:, hp * P:(hp + 1) * P],
                            vps[sub][:, hp * 2 * HE:(hp + 1) * 2 * HE],
                            start=(sub == 0), stop=(sub == NSUB - 1))
                    # good quadrants: rows 0:64 cols 0:HE (head 2hp),
                    # rows 64:128 cols HE:2HE (head 2hp+1)
                    a0 = (2 * hp) * HE
                    nc.vector.tensor_tensor(
                        kv_sb[:, a0:a0 + HE], kv_sb[:, a0:a0 + HE],
                        kvt[0:HD, 0:HE], Alu.add)
                    a1 = (2 * hp + 1) * HE
                    nc.vector.tensor_tensor(
                        kv_sb[:, a1:a1 + HE], kv_sb[:, a1:a1 + HE],
                        kvt[HD:P, HE:2 * HE], Alu.add)

            for ci in range(NCHUNK):
                do_chunk(ci, xt0 if ci == 0 else load_xt(ci))

            # ---- build block-diagonal kv (with rank-1 bv fix) + den cols ----
            for ct in range(NCT):
                for half in range(2):
                    h = 2 * ct + half
                    dst = kvbn[ct][half * HD:(half + 1) * HD,
                                   half * HD:(half + 1) * HD]
                    ks_col = kv_sb[:, h * HE + HD:h * HE + HD + 1]
                    # kv_fixed = bv_h * ksum_h + kv_h   (rank-1 bias fold)
                    nc.vector.scalar_tensor_tensor(
                        dst, bv_rep[0:HD, h * HD:(h + 1) * HD], ks_col,
                        kv_sb[:, h * HE:h * HE + HD], Alu.mult, Alu.add)
                    nc.vector.tensor_copy(
                        kvbd[ct][half * HD:(half + 1) * HD, half:half + 1],
                        ks_col)

            # ---- tail: numerator/denominator + divide + store ----
            for cj in range(NCHUNK):
                tok0 = cj * CHUNK
                for sub in range(NSUB):
                    pn = pps.tile([P, CG], F32, tag="pps", name="pn")
                    pd = sps.tile([P, 2 * NCT], F32, tag="sps", name="pd")
                    for ct in range(NCT):
                        q0 = (cj * NCT + ct) * CHUNK + sub * P
                        nc.tensor.matmul(
                            pn[:, ct * P:(ct + 1) * P],
                            qft_all[:, q0:q0 + P], kvbn[ct][:],
                            start=True, stop=True)
                        nc.tensor.matmul(
                            pd[:, ct * 2:(ct + 1) * 2],
                            qft_all[:, q0:q0 + P], kvbd[ct][:],
                            start=True, stop=True)
                    rc = rcpool.tile([P, 2 * NCT], F32, tag="rc", name="rc")
                    nc.vector.reciprocal(rc[:], pd[:])
                    ot = outpool.tile([P, CG], BF, tag="out", name="osb")
                    rcb = rc[:].unsqueeze(2).broadcast_to(
                        [P, 2 * NCT, HD])
                    pn3 = pn[:].rearrange("p (h e) -> p h e", e=HD)
                    ot3 = ot[:].rearrange("p (h e) -> p h e", e=HD)
                    if sub % 2 == 0:
                        nc.vector.tensor_tensor(ot3, pn3, rcb, Alu.mult)
                    else:
                        # per-head scalar muls on ACT to offload DVE
                        for h in range(NH):
                            nc.scalar.mul(
                                ot[:, h * HD:(h + 1) * HD],
                                pn[:, h * HD:(h + 1) * HD],
                                rc[:, h:h + 1])
                    nc.sync.dma_start(
                        out_d[tok0 + sub * P:tok0 + (sub + 1) * P, :], ot[:])

    nc.compile()
    return nc


def _get_nc():
    global _CACHED_NC
    if _CACHED_NC is None:
        _CACHED_NC = _build()
    return _CACHED_NC


def _make_in_maps(hidden_states, Wq, bq, Wk, bk, Wv, bv):
    hs = np.asarray(hidden_states, np.float32)
    ones = np.ones((1, P), BF16)
    wq = np.asarray(Wq, np.float32).astype(BF16)
    wk = np.asarray(Wk, np.float32).astype(BF16)
    wv = np.asarray(Wv, np.float32).astype(BF16)
    bqf = np.asarray(bq, np.float32)
    bkf = np.asarray(bk, np.float32).astype(BF16)
    bvf = np.asarray(bv, np.float32)
    xts = [np.ascontiguousarray(hs[b].T).astype(BF16) for b in range(B)]
    in_maps = []
    for c in range(NCORES):
        b, g = divmod(c, 2)
        sl = slice(g * CG, (g + 1) * CG)
        in_maps.append({
            "xt": xts[b],
            "wq": np.ascontiguousarray(wq[:, sl]),
            "wk": np.ascontiguousarray(wk[:, sl]),
            "wv": np.ascontiguousarray(wv[:, sl]),
            "bq": np.ascontiguousarray(bqf[sl]),
            "bk": np.ascontiguousarray(bkf[sl]).reshape(1, CG),
            "bv": np.ascontiguousarray(bvf[sl]).reshape(1, CG),
            "onesb": ones,
        })
    return in_maps


def _run(in_maps, **kwargs):
    from concourse.bass_utils import run_bass_kernel_spmd
    nc = _get_nc()
    return run_bass_kernel_spmd(nc, in_maps, core_ids=list(range(NCORES)),
                                **kwargs)


def _assemble(results):
    out = np.empty((B, S, D), np.float32)
    for c in range(NCORES):
        b, g = divmod(c, 2)
        out[b, :, g * CG:(g + 1) * CG] = np.asarray(
            results[c]["out"], dtype=np.float32)
    return out


def kernel(hidden_states, Wq, bq, Wk, bk, Wv, bv):
    in_maps = _make_in_maps(hidden_states, Wq, bq, Wk, bk, Wv, bv)
    res = _run(in_maps)
    return _assemble(res.results)


# revision 6
# speedup vs baseline: 1.2445x; 1.0463x over previous
"""Trainium2 Bass kernel for BertLinearSelfAttention (linear attention).

Reference computation (per batch b, head h):
    q,k,v = X @ W{q,k,v} + b{q,k,v}            # [S, D] -> heads of 64
    qf, kf = elu(q)+1, elu(k)+1                # = min(exp(x),1) + max(x,0)
    kv[d,e]  = sum_s kf[s,d] v[s,e]            # [64, 64]
    ksum[d]  = sum_s kf[s,d]
    out[s,e] = (sum_d qf[s,d] kv[d,e]) / (sum_d qf[s,d] ksum[d])

Sharding: 8 cores = (4 batches) x (2 head-groups of 8 heads / 512 proj cols).

All matmuls in bf16 (1 col/cycle, same PE rate as fp32r; the 2e-2 rel-err
gate leaves ~50x headroom over bf16's ~0.5% noise). Single pass over X:
per 512-token chunk compute k/v/q projections + feature maps, accumulate
kv/ksum, and stash q-features (bf16, 4.2MB) in SBUF. The v bias is folded
into kv afterwards as the rank-1 update ksum x bv, so the v path needs only
a PSUM->SBUF copy (on ACT). Tail: per chunk, block-diagonal num/den matmuls
+ reciprocal + broadcast multiply (split DVE/ACT/GPS) + bf16 store.
"""

import os
import sys

import numpy as np
import ml_dtypes

_REPO = "/opt/trn_rl_repo"
if os.path.isdir(_REPO) and _REPO not in sys.path:
    sys.path.insert(0, _REPO)

B, S, D, H, HD = 4, 4096, 1024, 16, 64
NCORES = 8
CG = 512            # projection columns per core (8 heads)
NH = CG // HD       # 8 heads per core
HE = HD + 2         # head cols incl ksum column + pad
CHUNK = 512         # tokens per chunk
NSUB = CHUNK // 128     # 4 token sub-tiles per chunk
NCHUNK = S // CHUNK     # 8 chunks
NKT = D // 128          # 8 contraction tiles
P = 128
NCT = CG // P           # 4 column tiles (2 heads each)

BF16 = ml_dtypes.bfloat16

_CACHED_NC = None


def _build():
    import concourse.tile as tile
    from concourse import bacc, mybir
    from contextlib import ExitStack

    F32 = mybir.dt.float32
    BF = mybir.dt.bfloat16
    Alu = mybir.AluOpType
    Act = mybir.ActivationFunctionType

    nc = bacc.Bacc("TRN2", target_bir_lowering=False, debug=False,
                   num_devices=NCORES)

    xt_d = nc.dram_tensor("xt", [D, S], BF, kind="ExternalInput").ap()
    wk_d = nc.dram_tensor("wk", [D, CG], BF, kind="ExternalInput").ap()
    wv_d = nc.dram_tensor("wv", [D, CG], BF, kind="ExternalInput").ap()
    wq_d = nc.dram_tensor("wq", [D, CG], BF, kind="ExternalInput").ap()
    bq_d = nc.dram_tensor("bq", [CG], F32, kind="ExternalInput").ap()
    bk_d = nc.dram_tensor("bk", [1, CG], BF, kind="ExternalInput").ap()
    bv_d = nc.dram_tensor("bv", [1, NH * HD], F32, kind="ExternalInput").ap()
    ones_d = nc.dram_tensor("onesb", [1, P], BF, kind="ExternalInput").ap()
    out_d = nc.dram_tensor("out", [S, CG], BF, kind="ExternalOutput").ap()

    with tile.TileContext(nc) as tc:
        with ExitStack() as ctx:
            const = ctx.enter_context(tc.tile_pool(name="const", bufs=1))
            wpool = ctx.enter_context(tc.tile_pool(name="wpool", bufs=1))
            xt0pool = ctx.enter_context(tc.tile_pool(name="xt0pool", bufs=1))
            xtpool = ctx.enter_context(tc.tile_pool(name="xtpool", bufs=3))

            # chunk 0: per-ktile loads for the earliest possible first matmul;
            # wk[kt0] goes first on the same (sync) queue.
            w_sb = {}
            for nm in ("k", "v", "q"):
                w_sb[nm] = wpool.tile([P, NKT * CG], BF, tag=f"w{nm}",
                                      name=f"w{nm}")
            nc.sync.dma_start(w_sb["k"][:, 0:CG], wk_d[0:P, :])
            xt0 = []
            for kt in range(NKT):
                t = xt0pool.tile([P, CHUNK], BF, tag=f"xt0_{kt}",
                                 name=f"xt0_{kt}")
                nc.sync.dma_start(t[:], xt_d[kt * P:(kt + 1) * P, 0:CHUNK])
                xt0.append(t)

            # remaining weights on the gpsimd queue
            for nm, wd in (("k", wk_d), ("v", wv_d), ("q", wq_d)):
                for kt in range(NKT):
                    if nm == "k" and kt == 0:
                        continue
                    nc.gpsimd.dma_start(w_sb[nm][:, kt * CG:(kt + 1) * CG],
                                        wd[kt * P:(kt + 1) * P, :])

            # chunks 1..7: one DMA per chunk ([128, kt-major 4096])
            def load_xt(ci):
                tok0 = ci * CHUNK
                t = xtpool.tile([P, NKT * CHUNK], BF, tag="xt", name="xt")
                nc.sync.dma_start(
                    t[:].rearrange("p (k c) -> p k c", k=NKT),
                    xt_d[:, tok0:tok0 + CHUNK].rearrange(
                        "(k p) c -> p k c", p=P))
                return t

            # ---- small constants ----
            ones_r = const.tile([1, P], BF, tag="onesb")
            nc.sync.dma_start(ones_r[:], ones_d[:])
            bk_r = const.tile([1, CG], BF, tag="bkr")
            nc.sync.dma_start(bk_r[:], bk_d[:])
            bq_sb = const.tile([P, NCT], F32, tag="bqsb")
            nc.sync.dma_start(bq_sb[:], bq_d.rearrange("(c p) -> p c", p=P))
            bv_sb = const.tile([1, NH * HD], F32, tag="bv32")
            nc.sync.dma_start(bv_sb[:], bv_d[:])
            bv_rep = const.tile([P, NH * HD], F32, tag="bvrep")
            nc.gpsimd.partition_broadcast(bv_rep[:], bv_sb[:])

            # kv accumulator (SBUF f32): per head [64, HE] (ksum in col HD)
            kv_sb = wpool.tile([HD, NH * HE], F32, tag="kvsb")
            nc.vector.memset(kv_sb[:], 0.0)

            # block-diagonal num weights [128,128] per ct + den cols [128,2]
            kvbn = [wpool.tile([P, P], BF, tag=f"kvbn{i}", name=f"kvbn{i}")
                    for i in range(NCT)]
            kvbd = [wpool.tile([P, 2], BF, tag=f"kvbd{i}", name=f"kvbd{i}")
                    for i in range(NCT)]
            for t in kvbn + kvbd:
                nc.vector.memset(t[:], 0.0)

            # persistent V' tiles (2 chunks' worth): tail cols preset once
            vp_tiles = [wpool.tile([P, NH * HE], BF, tag=f"vp{i}",
                                   name=f"vp{i}") for i in range(2 * NSUB)]
            for t in vp_tiles:
                nc.vector.memset(
                    t[:].rearrange("p (h e) -> p h e", e=HE)[:, :, HD:], 0.0)
                nc.vector.memset(
                    t[:].rearrange("p (h e) -> p h e", e=HE)[:, :, HD:HD + 1],
                    1.0)

            # q-feature store for the whole sequence (bf16, 4.2MB)
            qft_all = wpool.tile([P, NCHUNK * NCT * CHUNK], BF, tag="qft")

            kfpool = ctx.enter_context(tc.tile_pool(name="kfpool", bufs=6))
            tmp = ctx.enter_context(tc.tile_pool(name="tmp", bufs=10))
            outpool = ctx.enter_context(tc.tile_pool(name="outp", bufs=6))
            rcpool = ctx.enter_context(tc.tile_pool(name="rcp", bufs=8))
            pps = ctx.enter_context(
                tc.tile_pool(name="pps", bufs=5, space="PSUM"))
            sps = ctx.enter_context(
                tc.tile_pool(name="sps", bufs=3, space="PSUM"))

            def build_kvblocks():
                # block-diagonal kv (with rank-1 bv fix) + den columns
                for ct in range(NCT):
                    for half in range(2):
                        h = 2 * ct + half
                        dst = kvbn[ct][half * HD:(half + 1) * HD,
                                       half * HD:(half + 1) * HD]
                        ks_col = kv_sb[:, h * HE + HD:h * HE + HD + 1]
                        # kv_fixed = bv_h * ksum_h + kv_h  (rank-1 bias fold)
                        nc.vector.scalar_tensor_tensor(
                            dst, bv_rep[0:HD, h * HD:(h + 1) * HD], ks_col,
                            kv_sb[:, h * HE:h * HE + HD], Alu.mult, Alu.add)
                        nc.vector.tensor_copy(
                            kvbd[ct][half * HD:(half + 1) * HD,
                                     half:half + 1],
                            ks_col)

            def do_chunk(ci, xt):
                kfs = []
                # ---- k projection + feature map ----
                for sub in range(NSUB):
                    ps = pps.tile([P, CG], F32, tag="pps", name="kps")
                    for kt in range(NKT):
                        nc.tensor.matmul(
                            ps[:],
                            xt(kt)[:, sub * P:(sub + 1) * P],
                            w_sb["k"][:, kt * CG:(kt + 1) * CG],
                            start=(kt == 0), stop=False)
                    nc.tensor.matmul(ps[:], ones_r[:], bk_r[:],
                                     start=False, stop=True)
                    e = tmp.tile([P, CG], BF, tag="t", name="t_e")
                    nc.scalar.activation(e[:], ps[:], Act.Exp)
                    m = tmp.tile([P, CG], BF, tag="t", name="t_m")
                    nc.vector.tensor_scalar(m[:], e[:], 1.0, None, Alu.min)
                    kf = kfpool.tile([P, CG], BF, tag="kf", name="kf")
                    # kf = max(ps,0) + m
                    nc.vector.scalar_tensor_tensor(
                        kf[:], ps[:], 0.0, m[:], Alu.max, Alu.add)
                    kfs.append(kf)
                # ---- v projection (no bias; folded into kv later) ----
                vps = []
                for sub in range(NSUB):
                    ps = pps.tile([P, CG], F32, tag="pps", name="vps")
                    for kt in range(NKT):
                        nc.tensor.matmul(
                            ps[:],
                            xt(kt)[:, sub * P:(sub + 1) * P],
                            w_sb["v"][:, kt * CG:(kt + 1) * CG],
                            start=(kt == 0), stop=(kt == NKT - 1))
                    vp = vp_tiles[(ci % 2) * NSUB + sub]
                    nc.scalar.copy(
                        vp[:].rearrange("p (h e) -> p h e", e=HE)[:, :, :HD],
                        ps[:].rearrange("p (h e) -> p h e", e=HD))
                    vps.append(vp)

                def do_q():
                    for ct in range(NCT):
                        ps = pps.tile([P, CHUNK], F32, tag="pps", name="qps")
                        for kt in range(NKT):
                            nc.tensor.matmul(
                                ps[:],
                                w_sb["q"][:, kt * CG + ct * P:
                                          kt * CG + (ct + 1) * P],
                                xt(kt)[:],
                                start=(kt == 0), stop=(kt == NKT - 1))
                        bcol = bq_sb[:, ct:ct + 1]
                        e = tmp.tile([P, CHUNK], BF, tag="t", name="t_qe")
                        nc.scalar.activation(e[:], ps[:], Act.Exp, bias=bcol)
                        m = tmp.tile([P, CHUNK], BF, tag="t", name="t_qm")
                        nc.vector.tensor_scalar(m[:], e[:], 1.0, None,
                                                Alu.min)
                        r = tmp.tile([P, CHUNK], BF, tag="t", name="t_qr")
                        nc.vector.tensor_scalar(
                            r[:], ps[:], bcol, 0.0, Alu.add, Alu.max)
                        q0 = (ci * NCT + ct) * CHUNK
                        nc.vector.tensor_tensor(
                            qft_all[:, q0:q0 + CHUNK], m[:], r[:], Alu.add)

                def do_kv():
                    # kv accumulation (head pairs: M=128, N=2*HE)
                    for hp in range(NH // 2):
                        kvt = sps.tile([P, 2 * HE], F32, tag="sps",
                                       name="kvt")
                        for sub in range(NSUB):
                            nc.tensor.matmul(
                                kvt[:],
                                kfs[sub][:, hp * P:(hp + 1) * P],
                                vps[sub][:, hp * 2 * HE:(hp + 1) * 2 * HE],
                                start=(sub == 0), stop=(sub == NSUB - 1))
                        # good quadrants: rows 0:64 cols 0:HE (head 2hp),
                        # rows 64:128 cols HE:2HE (head 2hp+1)
                        a0 = (2 * hp) * HE
                        nc.vector.tensor_tensor(
                            kv_sb[:, a0:a0 + HE], kv_sb[:, a0:a0 + HE],
                            kvt[0:HD, 0:HE], Alu.add)
                        a1 = (2 * hp + 1) * HE
                        nc.vector.tensor_tensor(
                            kv_sb[:, a1:a1 + HE], kv_sb[:, a1:a1 + HE],
                            kvt[HD:P, HE:2 * HE], Alu.add)

                if ci < NCHUNK - 1:
                    do_q()
                    do_kv()
                else:
                    # last chunk: kv first so the kv-block build (DVE)
                    # overlaps the final q matmuls (PE)
                    do_kv()
                    build_kvblocks()
                    do_q()

            xt0_f = lambda kt: xt0[kt][:]
            do_chunk(0, xt0_f)
            for ci in range(1, NCHUNK):
                xtb = load_xt(ci)
                do_chunk(ci, lambda kt, t=xtb: t[:, kt * CHUNK:
                                                 (kt + 1) * CHUNK])

            # ---- tail: numerator/denominator + divide + store ----
            for cj in range(NCHUNK):
                tok0 = cj * CHUNK
                for sub in range(NSUB):
                    pn = pps.tile([P, CG], F32, tag="pps", name="pn")
                    pd = sps.tile([P, 2 * NCT], F32, tag="sps", name="pd")
                    for ct in range(NCT):
                        q0 = (cj * NCT + ct) * CHUNK + sub * P
                        nc.tensor.matmul(
                            pn[:, ct * P:(ct + 1) * P],
                            qft_all[:, q0:q0 + P], kvbn[ct][:],
                            start=True, stop=True)
                        nc.tensor.matmul(
                            pd[:, ct * 2:(ct + 1) * 2],
                            qft_all[:, q0:q0 + P], kvbd[ct][:],
                            start=True, stop=True)
                    rc = rcpool.tile([P, 2 * NCT], F32, tag="rc", name="rc")
                    nc.vector.reciprocal(rc[:], pd[:])
                    ot = outpool.tile([P, CG], BF, tag="out", name="osb")
                    rcb = rc[:].unsqueeze(2).broadcast_to(
                        [P, 2 * NCT, HD])
                    pn3 = pn[:].rearrange("p (h e) -> p h e", e=HD)
                    ot3 = ot[:].rearrange("p (h e) -> p h e", e=HD)
                    if sub < 3:
                        nc.vector.tensor_tensor(ot3, pn3, rcb, Alu.mult)
                    else:
                        # per-head scalar muls on ACT
                        for h in range(NH):
                            nc.scalar.mul(
                                ot[:, h * HD:(h + 1) * HD],
                                pn[:, h * HD:(h + 1) * HD],
                                rc[:, h:h + 1])
                    nc.sync.dma_start(
                        out_d[tok0 + sub * P:tok0 + (sub + 1) * P, :], ot[:])

    nc.compile()
    return nc


def _get_nc():
    global _CACHED_NC
    if _CACHED_NC is None:
        _CACHED_NC = _build()
    return _CACHED_NC


def _make_in_maps(hidden_states, Wq, bq, Wk, bk, Wv, bv):
    hs = np.asarray(hidden_states, np.float32)
    ones = np.ones((1, P), BF16)
    wq = np.asarray(Wq, np.float32).astype(BF16)
    wk = np.asarray(Wk, np.float32).astype(BF16)
    wv = np.asarray(Wv, np.float32).astype(BF16)
    bqf = np.asarray(bq, np.float32)
    bkf = np.asarray(bk, np.float32).astype(BF16)
    bvf = np.asarray(bv, np.float32)
    xts = [np.ascontiguousarray(hs[b].T).astype(BF16) for b in range(B)]
    in_maps = []
    for c in range(NCORES):
        b, g = divmod(c, 2)
        sl = slice(g * CG, (g + 1) * CG)
        in_maps.append({
            "xt": xts[b],
            "wq": np.ascontiguousarray(wq[:, sl]),
            "wk": np.ascontiguousarray(wk[:, sl]),
            "wv": np.ascontiguousarray(wv[:, sl]),
            "bq": np.ascontiguousarray(bqf[sl]),
            "bk": np.ascontiguousarray(bkf[sl]).reshape(1, CG),
            "bv": np.ascontiguousarray(bvf[sl]).reshape(1, CG),
            "onesb": ones,
        })
    return in_maps


def _run(in_maps, **kwargs):
    from concourse.bass_utils import run_bass_kernel_spmd
    nc = _get_nc()
    return run_bass_kernel_spmd(nc, in_maps, core_ids=list(range(NCORES)),
                                **kwargs)


def _assemble(results):
    out = np.empty((B, S, D), np.float32)
    for c in range(NCORES):
        b, g = divmod(c, 2)
        out[b, :, g * CG:(g + 1) * CG] = np.asarray(
            results[c]["out"], dtype=np.float32)
    return out


def kernel(hidden_states, Wq, bq, Wk, bk, Wv, bv):
    in_maps = _make_in_maps(hidden_states, Wq, bq, Wk, bk, Wv, bv)
    res = _run(in_maps)
    return _assemble(res.results)


# revision 14
# speedup vs baseline: 1.2692x; 1.0198x over previous
"""Trainium2 Bass kernel for BertLinearSelfAttention (linear attention).

Reference computation (per batch b, head h):
    q,k,v = X @ W{q,k,v} + b{q,k,v}            # [S, D] -> heads of 64
    qf, kf = elu(q)+1, elu(k)+1                # = min(exp(x),1) + max(x,0)
    kv[d,e]  = sum_s kf[s,d] v[s,e]            # [64, 64]
    ksum[d]  = sum_s kf[s,d]
    out[s,e] = (sum_d qf[s,d] kv[d,e]) / (sum_d qf[s,d] ksum[d])

Sharding: 8 cores = (4 batches) x (2 head-groups of 8 heads / 512 proj cols).

All matmuls in bf16 (1 col/cycle, same PE rate as fp32r; the 2e-2 rel-err
gate leaves ~50x headroom over bf16's ~0.5% noise). Single pass over X:
per 512-token chunk compute k/v/q projections + feature maps, accumulate
kv/ksum, and stash q-features (bf16, 4.2MB) in SBUF. The v bias is folded
into kv afterwards as the rank-1 update ksum x bv, so the v path needs only
a PSUM->SBUF copy (on ACT). Tail: per chunk, block-diagonal num/den matmuls
+ reciprocal + broadcast multiply (split DVE/ACT/GPS) + bf16 store.
"""

import os
import sys

import numpy as np
import ml_dtypes

_REPO = "/opt/trn_rl_repo"
if os.path.isdir(_REPO) and _REPO not in sys.path:
    sys.path.insert(0, _REPO)

B, S, D, H, HD = 4, 4096, 1024, 16, 64
NCORES = 8
CG = 512            # projection columns per core (8 heads)
NH = CG // HD       # 8 heads per core
HE = HD + 2         # head cols incl ksum column + pad
CHUNK = 512         # tokens per chunk
NSUB = CHUNK // 128     # 4 token sub-tiles per chunk
NCHUNK = S // CHUNK     # 8 chunks
NKT = D // 128          # 8 contraction tiles
P = 128
NCT = CG // P           # 4 column tiles (2 heads each)

BF16 = ml_dtypes.bfloat16

_CACHED_NC = None


def _build():
    import concourse.tile as tile
    from concourse import bacc, mybir
    from contextlib import ExitStack

    F32 = mybir.dt.float32
    BF = mybir.dt.bfloat16
    Alu = mybir.AluOpType
    Act = mybir.ActivationFunctionType

    nc = bacc.Bacc("TRN2", target_bir_lowering=False, debug=False,
                   num_devices=NCORES)

    xt_d = nc.dram_tensor("xt", [D, S], BF, kind="ExternalInput").ap()
    wk_d = nc.dram_tensor("wk", [D, CG], BF, kind="ExternalInput").ap()
    wv_d = nc.dram_tensor("wv", [D, CG], BF, kind="ExternalInput").ap()
    wq_d = nc.dram_tensor("wq", [D, CG], BF, kind="ExternalInput").ap()
    bq_d = nc.dram_tensor("bq", [CG], F32, kind="ExternalInput").ap()
    bk_d = nc.dram_tensor("bk", [1, CG], BF, kind="ExternalInput").ap()
    bv_d = nc.dram_tensor("bv", [1, NH * HD], F32, kind="ExternalInput").ap()
    out_d = nc.dram_tensor("out", [S, CG], BF, kind="ExternalOutput").ap()

    with tile.TileContext(nc) as tc:
        with ExitStack() as ctx:
            const = ctx.enter_context(tc.tile_pool(name="const", bufs=1))
            wpool = ctx.enter_context(tc.tile_pool(name="wpool", bufs=1))
            xt0pool = ctx.enter_context(tc.tile_pool(name="xt0pool", bufs=1))
            xtpool = ctx.enter_context(tc.tile_pool(name="xtpool", bufs=3))

            # chunk 0: per-ktile loads for the earliest possible first matmul;
            # the first half of wk goes first on the same (sync) queue.
            w_sb = {}
            for nm in ("k", "v", "q"):
                w_sb[nm] = wpool.tile([P, NKT * CG], BF, tag=f"w{nm}",
                                      name=f"w{nm}")

            def load_w(nm, wd, kt0, kt1, eng):
                eng.dma_start(
                    w_sb[nm][:, kt0 * CG:kt1 * CG].rearrange(
                        "p (k c) -> p k c", k=kt1 - kt0),
                    wd[kt0 * P:kt1 * P, :].rearrange("(k p) c -> p k c", p=P))

            load_w("k", wk_d, 0, NKT // 2, nc.sync)
            xt0 = []
            for kt in range(NKT):
                t = xt0pool.tile([P, CHUNK], BF, tag=f"xt0_{kt}",
                                 name=f"xt0_{kt}")
                nc.sync.dma_start(t[:], xt_d[kt * P:(kt + 1) * P, 0:CHUNK])
                xt0.append(t)

            # remaining weights on the gpsimd queue (one DMA each)
            load_w("k", wk_d, NKT // 2, NKT, nc.gpsimd)
            load_w("v", wv_d, 0, NKT, nc.gpsimd)
            load_w("q", wq_d, 0, NKT, nc.gpsimd)

            # chunks 1..7: one DMA per chunk ([128, kt-major 4096])
            def load_xt(ci):
                tok0 = ci * CHUNK
                t = xtpool.tile([P, NKT * CHUNK], BF, tag="xt", name="xt")
                nc.sync.dma_start(
                    t[:].rearrange("p (k c) -> p k c", k=NKT),
                    xt_d[:, tok0:tok0 + CHUNK].rearrange(
                        "(k p) c -> p k c", p=P))
                return t

            # ---- small constants ----
            bk_r = const.tile([1, CG], BF, tag="bkr")
            nc.sync.dma_start(bk_r[:], bk_d[:])
            bk_rep = const.tile([P, CG], BF, tag="bkrep")
            nc.gpsimd.partition_broadcast(bk_rep[:], bk_r[:])
            bq_sb = const.tile([P, NCT], F32, tag="bqsb")
            nc.sync.dma_start(bq_sb[:], bq_d.rearrange("(c p) -> p c", p=P))
            bv_sb = const.tile([1, NH * HD], F32, tag="bv32")
            nc.sync.dma_start(bv_sb[:], bv_d[:])
            bv_rep = const.tile([P, NH * HD], F32, tag="bvrep")
            nc.gpsimd.partition_broadcast(bv_rep[:], bv_sb[:])

            # kv accumulator (SBUF f32): per head [64, HE] (ksum in col HD)
            kv_sb = wpool.tile([HD, NH * HE], F32, tag="kvsb")
            nc.vector.memset(kv_sb[:], 0.0)

            # block-diagonal num weights [128,128] per ct + den cols [128,2]
            kvbn = [wpool.tile([P, P], BF, tag=f"kvbn{i}", name=f"kvbn{i}")
                    for i in range(NCT)]
            kvbd = [wpool.tile([P, 2], BF, tag=f"kvbd{i}", name=f"kvbd{i}")
                    for i in range(NCT)]
            for t in kvbn + kvbd:
                nc.vector.memset(t[:], 0.0)

            # persistent V' tiles (2 chunks' worth): tail cols preset once
            vp_tiles = [wpool.tile([P, NH * HE], BF, tag=f"vp{i}",
                                   name=f"vp{i}") for i in range(2 * NSUB)]
            for t in vp_tiles:
                nc.vector.memset(
                    t[:].rearrange("p (h e) -> p h e", e=HE)[:, :, HD:], 0.0)
                nc.vector.memset(
                    t[:].rearrange("p (h e) -> p h e", e=HE)[:, :, HD:HD + 1],
                    1.0)

            # q-feature store for the whole sequence (bf16, 4.2MB)
            qft_all = wpool.tile([P, NCHUNK * NCT * CHUNK], BF, tag="qft")

            kfpool = ctx.enter_context(tc.tile_pool(name="kfpool", bufs=6))
            tmp = ctx.enter_context(tc.tile_pool(name="tmp", bufs=10))
            outpool = ctx.enter_context(tc.tile_pool(name="outp", bufs=6))
            rcpool = ctx.enter_context(tc.tile_pool(name="rcp", bufs=8))
            pps = ctx.enter_context(
                tc.tile_pool(name="pps", bufs=6, space="PSUM"))
            sps = ctx.enter_context(
                tc.tile_pool(name="sps", bufs=2, space="PSUM"))

            def build_kvblocks():
                # block-diagonal kv (with rank-1 bv fix) + den columns
                for ct in range(NCT):
                    for half in range(2):
                        h = 2 * ct + half
                        dst = kvbn[ct][half * HD:(half + 1) * HD,
                                       half * HD:(half + 1) * HD]
                        ks_col = kv_sb[:, h * HE + HD:h * HE + HD + 1]
                        # kv_fixed = bv_h * ksum_h + kv_h  (rank-1 bias fold)
                        nc.vector.scalar_tensor_tensor(
                            dst, bv_rep[0:HD, h * HD:(h + 1) * HD], ks_col,
                            kv_sb[:, h * HE:h * HE + HD], Alu.mult, Alu.add)
                        nc.vector.tensor_copy(
                            kvbd[ct][half * HD:(half + 1) * HD,
                                     half:half + 1],
                            ks_col)

            def do_chunk(ci, xt):
                kfs = []
                # ---- k projection + feature map (bias on DVE) ----
                for sub in range(NSUB):
                    ps = pps.tile([P, CG], F32, tag="pps", name="kps")
                    for kt in range(NKT):
                        nc.tensor.matmul(
                            ps[:],
                            xt(kt)[:, sub * P:(sub + 1) * P],
                            w_sb["k"][:, kt * CG:(kt + 1) * CG],
                            start=(kt == 0), stop=(kt == NKT - 1))
                    t = tmp.tile([P, CG], BF, tag="t", name="t_kb")
                    nc.vector.tensor_tensor(t[:], ps[:], bk_rep[:], Alu.add)
                    e = tmp.tile([P, CG], BF, tag="t", name="t_e")
                    nc.scalar.activation(e[:], t[:], Act.Exp)
                    m = tmp.tile([P, CG], BF, tag="t", name="t_m")
                    nc.vector.tensor_scalar(m[:], e[:], 1.0, None, Alu.min)
                    kf = kfpool.tile([P, CG], BF, tag="kf", name="kf")
                    # kf = max(t,0) + m
                    nc.vector.scalar_tensor_tensor(
                        kf[:], t[:], 0.0, m[:], Alu.max, Alu.add)
                    kfs.append(kf)
                # ---- v projection (no bias; folded into kv later) ----
                vps = []
                for sub in range(NSUB):
                    ps = pps.tile([P, CG], F32, tag="pps", name="vps")
                    for kt in range(NKT):
                        nc.tensor.matmul(
                            ps[:],
                            xt(kt)[:, sub * P:(sub + 1) * P],
                            w_sb["v"][:, kt * CG:(kt + 1) * CG],
                            start=(kt == 0), stop=(kt == NKT - 1))
                    vp = vp_tiles[(ci % 2) * NSUB + sub]
                    nc.scalar.copy(
                        vp[:].rearrange("p (h e) -> p h e", e=HE)[:, :, :HD],
                        ps[:].rearrange("p (h e) -> p h e", e=HD))
                    vps.append(vp)

                def do_q():
                    for ct in range(NCT):
                        ps = pps.tile([P, CHUNK], F32, tag="pps", name="qps")
                        for kt in range(NKT):
                            nc.tensor.matmul(
                                ps[:],
                                w_sb["q"][:, kt * CG + ct * P:
                                          kt * CG + (ct + 1) * P],
                                xt(kt)[:],
                                start=(kt == 0), stop=(kt == NKT - 1))
                        bcol = bq_sb[:, ct:ct + 1]
                        e = tmp.tile([P, CHUNK], BF, tag="t", name="t_qe")
                        nc.scalar.activation(e[:], ps[:], Act.Exp, bias=bcol)
                        m = tmp.tile([P, CHUNK], BF, tag="t", name="t_qm")
                        nc.vector.tensor_scalar(m[:], e[:], 1.0, None,
                                                Alu.min)
                        r = tmp.tile([P, CHUNK], BF, tag="t", name="t_qr")
                        nc.vector.tensor_scalar(
                            r[:], ps[:], bcol, 0.0, Alu.add, Alu.max)
                        q0 = (ci * NCT + ct) * CHUNK
                        nc.vector.tensor_tensor(
                            qft_all[:, q0:q0 + CHUNK], m[:], r[:], Alu.add)

                def do_kv():
                    # kv accumulation (head pairs: M=128, N=2*HE)
                    for hp in range(NH // 2):
                        kvt = sps.tile([P, 2 * HE], F32, tag="sps",
                                       name="kvt")
                        for sub in range(NSUB):
                            nc.tensor.matmul(
                                kvt[:],
                                kfs[sub][:, hp * P:(hp + 1) * P],
                                vps[sub][:, hp * 2 * HE:(hp + 1) * 2 * HE],
                                start=(sub == 0), stop=(sub == NSUB - 1))
                        # good quadrants: rows 0:64 cols 0:HE (head 2hp),
                        # rows 64:128 cols HE:2HE (head 2hp+1)
                        a0 = (2 * hp) * HE
                        nc.vector.tensor_tensor(
                            kv_sb[:, a0:a0 + HE], kv_sb[:, a0:a0 + HE],
                            kvt[0:HD, 0:HE], Alu.add)
                        a1 = (2 * hp + 1) * HE
                        nc.vector.tensor_tensor(
                            kv_sb[:, a1:a1 + HE], kv_sb[:, a1:a1 + HE],
                            kvt[HD:P, HE:2 * HE], Alu.add)

                if ci < NCHUNK - 1:
                    do_q()
                    do_kv()
                else:
                    # last chunk: kv first so the kv-block build (DVE)
                    # overlaps the final q matmuls (PE)
                    do_kv()
                    build_kvblocks()
                    do_q()

            xt0_f = lambda kt: xt0[kt][:]
            do_chunk(0, xt0_f)
            for ci in range(1, NCHUNK):
                xtb = load_xt(ci)
                do_chunk(ci, lambda kt, t=xtb: t[:, kt * CHUNK:
                                                 (kt + 1) * CHUNK])

            # ---- tail: numerator/denominator + divide + store ----
            for cj in range(NCHUNK):
                tok0 = cj * CHUNK
                pd = sps.tile([P, NSUB * 2 * NCT], F32, tag="sps", name="pd")
                pns = []
                for sub in range(NSUB):
                    pn = pps.tile([P, CG], F32, tag="pps", name="pn")
                    for ct in range(NCT):
                        q0 = (cj * NCT + ct) * CHUNK + sub * P
                        nc.tensor.matmul(
                            pn[:, ct * P:(ct + 1) * P],
                            qft_all[:, q0:q0 + P], kvbn[ct][:],
                            start=True, stop=True)
                        nc.tensor.matmul(
                            pd[:, sub * 2 * NCT + ct * 2:
                               sub * 2 * NCT + (ct + 1) * 2],
                            qft_all[:, q0:q0 + P], kvbd[ct][:],
                            start=True, stop=True)
                    pns.append(pn)
                rc = rcpool.tile([P, NSUB * 2 * NCT], F32, tag="rc",
                                 name="rc")
                nc.vector.reciprocal(rc[:], pd[:])
                for sub in range(NSUB):
                    pn = pns[sub]
                    ot = outpool.tile([P, CG], BF, tag="out", name="osb")
                    rcs = rc[:, sub * 2 * NCT:(sub + 1) * 2 * NCT]
                    rcb = rcs.unsqueeze(2).broadcast_to([P, 2 * NCT, HD])
                    pn3 = pn[:].rearrange("p (h e) -> p h e", e=HD)
                    ot3 = ot[:].rearrange("p (h e) -> p h e", e=HD)
                    if sub < 3:
                        nc.vector.tensor_tensor(ot3, pn3, rcb, Alu.mult)
                    else:
                        # per-head scalar muls on ACT
                        for h in range(NH):
                            nc.scalar.mul(
                                ot[:, h * HD:(h + 1) * HD],
                                pn[:, h * HD:(h + 1) * HD],
                                rc[:, sub * 2 * NCT + h:
                                   sub * 2 * NCT + h + 1])
                    nc.sync.dma_start(
                        out_d[tok0 + sub * P:tok0 + (sub + 1) * P, :], ot[:])

    nc.compile()
    return nc


def _get_nc():
    global _CACHED_NC
    if _CACHED_NC is None:
        _CACHED_NC = _build()
    return _CACHED_NC


def _make_in_maps(hidden_states, Wq, bq, Wk, bk, Wv, bv):
    hs = np.asarray(hidden_states, np.float32)
    wq = np.asarray(Wq, np.float32).astype(BF16)
    wk = np.asarray(Wk, np.float32).astype(BF16)
    wv = np.asarray(Wv, np.float32).astype(BF16)
    bqf = np.asarray(bq, np.float32)
    bkf = np.asarray(bk, np.float32).astype(BF16)
    bvf = np.asarray(bv, np.float32)
    xts = [np.ascontiguousarray(hs[b].T).astype(BF16) for b in range(B)]
    in_maps = []
    for c in range(NCORES):
        b, g = divmod(c, 2)
        sl = slice(g * CG, (g + 1) * CG)
        in_maps.append({
            "xt": xts[b],
            "wq": np.ascontiguousarray(wq[:, sl]),
            "wk": np.ascontiguousarray(wk[:, sl]),
            "wv": np.ascontiguousarray(wv[:, sl]),
            "bq": np.ascontiguousarray(bqf[sl]),
            "bk": np.ascontiguousarray(bkf[sl]).reshape(1, CG),
            "bv": np.ascontiguousarray(bvf[sl]).reshape(1, CG),
        })
    return in_maps


def _run(in_maps, **kwargs):
    from concourse.bass_utils import run_bass_kernel_spmd
    nc = _get_nc()
    return run_bass_kernel_spmd(nc, in_maps, core_ids=list(range(NCORES)),
                                **kwargs)


def _assemble(results):
    out = np.empty((B, S, D), np.float32)
    for c in range(NCORES):
        b, g = divmod(c, 2)
        out[b, :, g * CG:(g + 1) * CG] = np.asarray(
            results[c]["out"], dtype=np.float32)
    return out


def kernel(hidden_states, Wq, bq, Wk, bk, Wv, bv):
    in_maps = _make_in_maps(hidden_states, Wq, bq, Wk, bk, Wv, bv)
    res = _run(in_maps)
    return _assemble(res.results)


# revision 22
# speedup vs baseline: 1.2779x; 1.0069x over previous
"""Trainium2 Bass kernel for BertLinearSelfAttention (linear attention).

Reference computation (per batch b, head h):
    q,k,v = X @ W{q,k,v} + b{q,k,v}            # [S, D] -> heads of 64
    qf, kf = elu(q)+1, elu(k)+1                # = min(exp(x),1) + max(x,0)
    kv[d,e]  = sum_s kf[s,d] v[s,e]            # [64, 64]
    ksum[d]  = sum_s kf[s,d]
    out[s,e] = (sum_d qf[s,d] kv[d,e]) / (sum_d qf[s,d] ksum[d])

Sharding: 8 cores = (4 batches) x (2 head-groups of 8 heads / 512 proj cols).

All matmuls in bf16 (1 col/cycle, same PE rate as fp32r; the 2e-2 rel-err
gate leaves ~50x headroom over bf16's ~0.5% noise). Single pass over X:
per 512-token chunk compute k/v/q projections + feature maps, accumulate
kv/ksum, and stash q-features (bf16, 4.2MB) in SBUF. The v bias is folded
into kv afterwards as the rank-1 update ksum x bv, so the v path needs only
a PSUM->SBUF copy (on ACT). Tail: per chunk, block-diagonal num/den matmuls
+ reciprocal + broadcast multiply (split DVE/ACT/GPS) + bf16 store.
"""

import os
import sys

import numpy as np
import ml_dtypes

_REPO = "/opt/trn_rl_repo"
if os.path.isdir(_REPO) and _REPO not in sys.path:
    sys.path.insert(0, _REPO)

B, S, D, H, HD = 4, 4096, 1024, 16, 64
NCORES = 8
CG = 512            # projection columns per core (8 heads)
NH = CG // HD       # 8 heads per core
HE = HD + 2         # head cols incl ksum column + pad
CHUNK = 512         # tokens per chunk
NSUB = CHUNK // 128     # 4 token sub-tiles per chunk
NCHUNK = S // CHUNK     # 8 chunks
NKT = D // 128          # 8 contraction tiles
P = 128
NCT = CG // P           # 4 column tiles (2 heads each)

BF16 = ml_dtypes.bfloat16

_CACHED_NC = None


def _build():
    import concourse.tile as tile
    from concourse import bacc, mybir
    from contextlib import ExitStack

    F32 = mybir.dt.float32
    BF = mybir.dt.bfloat16
    Alu = mybir.AluOpType
    Act = mybir.ActivationFunctionType

    nc = bacc.Bacc("TRN2", target_bir_lowering=False, debug=False,
                   num_devices=NCORES)

    xt_d = nc.dram_tensor("xt", [D, S], BF, kind="ExternalInput").ap()
    wk_d = nc.dram_tensor("wk", [D, CG], BF, kind="ExternalInput").ap()
    wv_d = nc.dram_tensor("wv", [D, CG], BF, kind="ExternalInput").ap()
    wq_d = nc.dram_tensor("wq", [D, CG], BF, kind="ExternalInput").ap()
    bq_d = nc.dram_tensor("bq", [CG], F32, kind="ExternalInput").ap()
    bk_d = nc.dram_tensor("bk", [1, CG], BF, kind="ExternalInput").ap()
    bv_d = nc.dram_tensor("bv", [1, NH * HD], F32, kind="ExternalInput").ap()
    out_d = nc.dram_tensor("out", [S, CG], BF, kind="ExternalOutput").ap()

    with tile.TileContext(nc) as tc:
        with ExitStack() as ctx:
            const = ctx.enter_context(tc.tile_pool(name="const", bufs=1))
            wpool = ctx.enter_context(tc.tile_pool(name="wpool", bufs=1))
            xt0pool = ctx.enter_context(tc.tile_pool(name="xt0pool", bufs=1))
            xtpool = ctx.enter_context(tc.tile_pool(name="xtpool", bufs=3))

            # chunk 0: per-ktile loads for the earliest possible first matmul;
            # the first half of wk goes first on the same (sync) queue.
            w_sb = {}
            for nm in ("k", "v", "q"):
                w_sb[nm] = wpool.tile([P, NKT * CG], BF, tag=f"w{nm}",
                                      name=f"w{nm}")

            def load_w(nm, wd, kt0, kt1, eng):
                eng.dma_start(
                    w_sb[nm][:, kt0 * CG:kt1 * CG].rearrange(
                        "p (k c) -> p k c", k=kt1 - kt0),
                    wd[kt0 * P:kt1 * P, :].rearrange("(k p) c -> p k c", p=P))

            # minimal first pieces so the first matmul can start ASAP
            load_w("k", wk_d, 0, 1, nc.sync)
            xt0 = []
            t = xt0pool.tile([P, CHUNK], BF, tag="xt0_0", name="xt0_0")
            nc.sync.dma_start(t[:], xt_d[0:P, 0:CHUNK])
            xt0.append(t)
            xt0r = xt0pool.tile([P, (NKT - 1) * CHUNK], BF, tag="xt0r",
                                name="xt0r")
            nc.sync.dma_start(
                xt0r[:].rearrange("p (k c) -> p k c", k=NKT - 1),
                xt_d[P:, 0:CHUNK].rearrange("(k p) c -> p k c", p=P))
            for kt in range(1, NKT):
                xt0.append(xt0r[:, (kt - 1) * CHUNK:kt * CHUNK])

            # remaining weights on the gpsimd queue (one DMA each)
            load_w("k", wk_d, 1, NKT, nc.gpsimd)
            load_w("v", wv_d, 0, NKT, nc.gpsimd)
            load_w("q", wq_d, 0, NKT, nc.gpsimd)

            # chunks 1..7: one DMA per chunk ([128, kt-major 4096])
            def load_xt(ci):
                tok0 = ci * CHUNK
                t = xtpool.tile([P, NKT * CHUNK], BF, tag="xt", name="xt")
                nc.sync.dma_start(
                    t[:].rearrange("p (k c) -> p k c", k=NKT),
                    xt_d[:, tok0:tok0 + CHUNK].rearrange(
                        "(k p) c -> p k c", p=P))
                return t

            # ---- small constants ----
            bk_r = const.tile([1, CG], BF, tag="bkr")
            nc.sync.dma_start(bk_r[:], bk_d[:])
            bk_rep = const.tile([P, CG], BF, tag="bkrep")
            nc.gpsimd.partition_broadcast(bk_rep[:], bk_r[:])
            bq_sb = const.tile([P, NCT], F32, tag="bqsb")
            nc.sync.dma_start(bq_sb[:], bq_d.rearrange("(c p) -> p c", p=P))
            bv_sb = const.tile([1, NH * HD], F32, tag="bv32")
            nc.sync.dma_start(bv_sb[:], bv_d[:])
            bv_rep = const.tile([P, NH * HD], F32, tag="bvrep")
            nc.gpsimd.partition_broadcast(bv_rep[:], bv_sb[:])

            # kv accumulator (SBUF f32): per head [64, HE] (ksum in col HD)
            kv_sb = wpool.tile([HD, NH * HE], F32, tag="kvsb")
            nc.vector.memset(kv_sb[:], 0.0)

            # block-diagonal num weights [128,128] per ct + den cols [128,2]
            kvbn = [wpool.tile([P, P], BF, tag=f"kvbn{i}", name=f"kvbn{i}")
                    for i in range(NCT)]
            kvbd = [wpool.tile([P, 2], BF, tag=f"kvbd{i}", name=f"kvbd{i}")
                    for i in range(NCT)]
            for t in kvbn + kvbd:
                nc.vector.memset(t[:], 0.0)

            # persistent V' tiles (2 chunks' worth): tail cols preset once
            vp_tiles = [wpool.tile([P, NH * HE], BF, tag=f"vp{i}",
                                   name=f"vp{i}") for i in range(2 * NSUB)]
            for t in vp_tiles:
                nc.vector.memset(
                    t[:].rearrange("p (h e) -> p h e", e=HE)[:, :, HD:], 0.0)
                nc.vector.memset(
                    t[:].rearrange("p (h e) -> p h e", e=HE)[:, :, HD:HD + 1],
                    1.0)

            # q-feature store for the whole sequence (bf16, 4.2MB)
            qft_all = wpool.tile([P, NCHUNK * NCT * CHUNK], BF, tag="qft")

            kfpool = ctx.enter_context(tc.tile_pool(name="kfpool", bufs=10))
            tmp = ctx.enter_context(tc.tile_pool(name="tmp", bufs=12))
            outpool = ctx.enter_context(tc.tile_pool(name="outp", bufs=6))
            rcpool = ctx.enter_context(tc.tile_pool(name="rcp", bufs=8))
            pps = ctx.enter_context(
                tc.tile_pool(name="pps", bufs=6, space="PSUM"))
            sps = ctx.enter_context(
                tc.tile_pool(name="sps", bufs=2, space="PSUM"))

            def build_kvblocks():
                # block-diagonal kv (with rank-1 bv fix) + den columns
                for ct in range(NCT):
                    for half in range(2):
                        h = 2 * ct + half
                        dst = kvbn[ct][half * HD:(half + 1) * HD,
                                       half * HD:(half + 1) * HD]
                        ks_col = kv_sb[:, h * HE + HD:h * HE + HD + 1]
                        # kv_fixed = bv_h * ksum_h + kv_h  (rank-1 bias fold)
                        nc.vector.scalar_tensor_tensor(
                            dst, bv_rep[0:HD, h * HD:(h + 1) * HD], ks_col,
                            kv_sb[:, h * HE:h * HE + HD], Alu.mult, Alu.add)
                        nc.vector.tensor_copy(
                            kvbd[ct][half * HD:(half + 1) * HD,
                                     half:half + 1],
                            ks_col)

            kf_c = {}
            vp_c = {}

            def do_k(ci, xt):
                kfs = []
                for sub in range(NSUB):
                    ps = pps.tile([P, CG], F32, tag="pps", name="kps")
                    for kt in range(NKT):
                        nc.tensor.matmul(
                            ps[:],
                            xt(kt)[:, sub * P:(sub + 1) * P],
                            w_sb["k"][:, kt * CG:(kt + 1) * CG],
                            start=(kt == 0), stop=(kt == NKT - 1))
                    t = tmp.tile([P, CG], BF, tag="t", name="t_kb")
                    nc.vector.tensor_tensor(t[:], ps[:], bk_rep[:], Alu.add)
                    e = tmp.tile([P, CG], BF, tag="t", name="t_e")
                    nc.scalar.activation(e[:], t[:], Act.Exp)
                    m = tmp.tile([P, CG], BF, tag="t", name="t_m")
                    nc.vector.tensor_scalar(m[:], e[:], 1.0, None, Alu.min)
                    kf = kfpool.tile([P, CG], BF, tag="kf", name="kf")
                    # kf = max(t,0) + m
                    nc.vector.scalar_tensor_tensor(
                        kf[:], t[:], 0.0, m[:], Alu.max, Alu.add)
                    kfs.append(kf)
                kf_c[ci] = kfs

            def do_v(ci, xt):
                vps = []
                for sub in range(NSUB):
                    ps = pps.tile([P, CG], F32, tag="pps", name="vps")
                    for kt in range(NKT):
                        nc.tensor.matmul(
                            ps[:],
                            xt(kt)[:, sub * P:(sub + 1) * P],
                            w_sb["v"][:, kt * CG:(kt + 1) * CG],
                            start=(kt == 0), stop=(kt == NKT - 1))
                    vp = vp_tiles[(ci % 2) * NSUB + sub]
                    nc.scalar.copy(
                        vp[:].rearrange("p (h e) -> p h e", e=HE)[:, :, :HD],
                        ps[:].rearrange("p (h e) -> p h e", e=HD))
                    vps.append(vp)
                vp_c[ci] = vps

            def do_q(ci, xt):
                for ct in range(NCT):
                    ps = pps.tile([P, CHUNK], F32, tag="pps", name="qps")
                    for kt in range(NKT):
                        nc.tensor.matmul(
                            ps[:],
                            w_sb["q"][:, kt * CG + ct * P:
                                      kt * CG + (ct + 1) * P],
                            xt(kt)[:],
                            start=(kt == 0), stop=(kt == NKT - 1))
                    bcol = bq_sb[:, ct:ct + 1]
                    e = tmp.tile([P, CHUNK], BF, tag="t", name="t_qe")
                    nc.scalar.activation(e[:], ps[:], Act.Exp, bias=bcol)
                    m = tmp.tile([P, CHUNK], BF, tag="t", name="t_qm")
                    nc.vector.tensor_scalar(m[:], e[:], 1.0, None, Alu.min)
                    r = tmp.tile([P, CHUNK], BF, tag="t", name="t_qr")
                    nc.vector.tensor_scalar(
                        r[:], ps[:], bcol, 0.0, Alu.add, Alu.max)
                    q0 = (ci * NCT + ct) * CHUNK
                    nc.vector.tensor_tensor(
                        qft_all[:, q0:q0 + CHUNK], m[:], r[:], Alu.add)

            def do_kv(ci):
                # kv accumulation (head pairs: M=128, N=2*HE)
                kfs, vps = kf_c.pop(ci), vp_c.pop(ci)
                for hp in range(NH // 2):
                    kvt = sps.tile([P, 2 * HE], F32, tag="sps", name="kvt")
                    for sub in range(NSUB):
                        nc.tensor.matmul(
                            kvt[:],
                            kfs[sub][:, hp * P:(hp + 1) * P],
                            vps[sub][:, hp * 2 * HE:(hp + 1) * 2 * HE],
                            start=(sub == 0), stop=(sub == NSUB - 1))
                    # good quadrants: rows 0:64 cols 0:HE (head 2hp),
                    # rows 64:128 cols HE:2HE (head 2hp+1)
                    a0 = (2 * hp) * HE
                    nc.vector.tensor_tensor(
                        kv_sb[:, a0:a0 + HE], kv_sb[:, a0:a0 + HE],
                        kvt[0:HD, 0:HE], Alu.add)
                    a1 = (2 * hp + 1) * HE
                    nc.vector.tensor_tensor(
                        kv_sb[:, a1:a1 + HE], kv_sb[:, a1:a1 + HE],
                        kvt[HD:P, HE:2 * HE], Alu.add)

            # chunk 0/1 interleaved at projection level: k needs only wk,
            # so both chunks' k-projections run while wv/wq still stream in
            xts = {0: lambda kt: xt0[kt]}
            xtb1 = load_xt(1)
            xts[1] = lambda kt: xtb1[:, kt * CHUNK:(kt + 1) * CHUNK]
            do_k(0, xts[0])
            do_k(1, xts[1])
            do_v(0, xts[0])
            do_v(1, xts[1])
            do_q(0, xts[0])
            do_kv(0)
            do_q(1, xts[1])
            do_kv(1)
            for ci in range(2, NCHUNK):
                xtb = load_xt(ci)
                xt = lambda kt, t=xtb: t[:, kt * CHUNK:(kt + 1) * CHUNK]
                do_k(ci, xt)
                do_v(ci, xt)
                if ci < NCHUNK - 1:
                    do_q(ci, xt)
                    do_kv(ci)
                else:
                    # last chunk: kv first so the kv-block build (DVE)
                    # overlaps the final q matmuls (PE)
                    do_kv(ci)
                    build_kvblocks()
                    do_q(ci, xt)

            # ---- tail: numerator/denominator + divide + store ----
            for cj in range(NCHUNK):
                tok0 = cj * CHUNK
                pd = sps.tile([P, NSUB * 2 * NCT], F32, tag="sps", name="pd")
                pns = []
                for sub in range(NSUB):
                    pn = pps.tile([P, CG], F32, tag="pps", name="pn")
                    for ct in range(NCT):
                        q0 = (cj * NCT + ct) * CHUNK + sub * P
                        nc.tensor.matmul(
                            pn[:, ct * P:(ct + 1) * P],
                            qft_all[:, q0:q0 + P], kvbn[ct][:],
                            start=True, stop=True)
                        nc.tensor.matmul(
                            pd[:, sub * 2 * NCT + ct * 2:
                               sub * 2 * NCT + (ct + 1) * 2],
                            qft_all[:, q0:q0 + P], kvbd[ct][:],
                            start=True, stop=True)
                    pns.append(pn)
                rc = rcpool.tile([P, NSUB * 2 * NCT], BF, tag="rc",
                                 name="rc")
                with nc.allow_low_precision(
                        reason="bf16 recip: denominators are O(1e3) sums"):
                    nc.vector.reciprocal(rc[:], pd[:])
                for sub in range(NSUB):
                    pn = pns[sub]
                    # PSUM f32 -> SBUF bf16 on ACT; mul runs all-bf16 on DVE
                    pnc = tmp.tile([P, CG], BF, tag="t", name="pnc")
                    nc.scalar.copy(pnc[:], pn[:])
                    ot = outpool.tile([P, CG], BF, tag="out", name="osb")
                    rcs = rc[:, sub * 2 * NCT:(sub + 1) * 2 * NCT]
                    rcb = rcs.unsqueeze(2).broadcast_to([P, 2 * NCT, HD])
                    pn3 = pnc[:].rearrange("p (h e) -> p h e", e=HD)
                    ot3 = ot[:].rearrange("p (h e) -> p h e", e=HD)
                    nc.vector.tensor_tensor(ot3, pn3, rcb, Alu.mult)
                    nc.sync.dma_start(
                        out_d[tok0 + sub * P:tok0 + (sub + 1) * P, :], ot[:])

    nc.compile()
    return nc


def _get_nc():
    global _CACHED_NC
    if _CACHED_NC is None:
        _CACHED_NC = _build()
    return _CACHED_NC


def _make_in_maps(hidden_states, Wq, bq, Wk, bk, Wv, bv):
    hs = np.asarray(hidden_states, np.float32)
    wq = np.asarray(Wq, np.float32).astype(BF16)
    wk = np.asarray(Wk, np.float32).astype(BF16)
    wv = np.asarray(Wv, np.float32).astype(BF16)
    bqf = np.asarray(bq, np.float32)
    bkf = np.asarray(bk, np.float32).astype(BF16)
    bvf = np.asarray(bv, np.float32)
    xts = [np.ascontiguousarray(hs[b].T).astype(BF16) for b in range(B)]
    in_maps = []
    for c in range(NCORES):
        b, g = divmod(c, 2)
        sl = slice(g * CG, (g + 1) * CG)
        in_maps.append({
            "xt": xts[b],
            "wq": np.ascontiguousarray(wq[:, sl]),
            "wk": np.ascontiguousarray(wk[:, sl]),
            "wv": np.ascontiguousarray(wv[:, sl]),
            "bq": np.ascontiguousarray(bqf[sl]),
            "bk": np.ascontiguousarray(bkf[sl]).reshape(1, CG),
            "bv": np.ascontiguousarray(bvf[sl]).reshape(1, CG),
        })
    return in_maps


def _run(in_maps, **kwargs):
    from concourse.bass_utils import run_bass_kernel_spmd
    nc = _get_nc()
    return run_bass_kernel_spmd(nc, in_maps, core_ids=list(range(NCORES)),
                                **kwargs)


def _assemble(results):
    out = np.empty((B, S, D), np.float32)
    for c in range(NCORES):
        b, g = divmod(c, 2)
        out[b, :, g * CG:(g + 1) * CG] = np.asarray(
            results[c]["out"], dtype=np.float32)
    return out


def kernel(hidden_states, Wq, bq, Wk, bk, Wv, bv):
    in_maps = _make_in_maps(hidden_states, Wq, bq, Wk, bk, Wv, bv)
    res = _run(in_maps)
    return _assemble(res.results)
